# revision 8
# baseline (speedup 1.0000x reference)
"""Bass/Trainium2 kernel for nn_Attention_37606733643961.

Full attention over B=4, S=56*56=3136, DIM=256, 8 heads of dim 32.
Sharding: 8 cores = (batch b, head-group g) with b = core//2, g = core%2.
Each core computes attention + projection-partial for its batch and its
4 heads; host sums the two projection partials per batch (row-split of
the c-contraction in the final matmul) and transposes back.

exp runs exactly on the Scalar engine (ACT) reading scores straight
from PSUM. Row-sums of exp ride the attn@v col-packed matmuls as
`ones` columns; normalization (reciprocal + multiply) happens
on-device before the projection. Matmuls are bf16 (fp32 PSUM accum).
A plain-mode "heater" matmul keeps the PE clock at 2.4 GHz (tiled
matmuls do not register as activity for the HAM clock gate).
"""

import sys

if "/opt/trn_rl_repo" not in sys.path:
    sys.path.insert(0, "/opt/trn_rl_repo")

import ml_dtypes
import numpy as np

import concourse.bacc as bacc
import concourse.mybir as mybir
from concourse.tile import TileContext
from concourse.bass_utils import run_bass_kernel_spmd

B, H, W, DIM = 4, 56, 56, 256
NUM_HEADS = 8
HEAD_DIM = 32
S = H * W                     # 3136
HL = 4                        # heads per core
SCALE = float(HEAD_DIM) ** -0.25

QB = 448                      # q-block (3136 = 7*448), <=512 (one PSUM bank)
NQB = S // QB                 # 7
KCH = [128] * 24 + [64]       # k-position chunks (3136 = 24*128 + 64)

F32 = mybir.dt.float32
BF16 = mybir.dt.bfloat16

_CACHE = {}


def build_nc():
    nc = bacc.Bacc(None, target_bir_lowering=False, debug=False)

    x = nc.dram_tensor("x", [S, DIM], BF16, kind="ExternalInput")
    wq = nc.dram_tensor("wq", [DIM, 128], BF16, kind="ExternalInput")
    wk = nc.dram_tensor("wk", [DIM, 128], BF16, kind="ExternalInput")
    wv = nc.dram_tensor("wv", [DIM, 128], BF16, kind="ExternalInput")
    # w_proj rows for this core's 4 heads, pre-split into head pairs
    wp = nc.dram_tensor("wp", [2, 64, DIM], BF16, kind="ExternalInput")
    outT = nc.dram_tensor("outT", [DIM, S], F32, kind="ExternalOutput")
    heat = nc.dram_tensor("heat", [32, 64], F32, kind="ExternalOutput")

    with TileContext(nc) as tc:
        with (
            tc.tile_pool(name="const", bufs=1) as constp,
            tc.tile_pool(name="wts", bufs=1) as wtp,
            tc.tile_pool(name="big", bufs=1) as bigp,
            tc.tile_pool(name="expst", bufs=8) as expp,
            tc.tile_pool(name="fin", bufs=3) as finp,
            tc.tile_pool(name="psc", bufs=3, space="PSUM") as pscp,
            tc.tile_pool(name="pout", bufs=1, space="PSUM") as poutp,
            tc.tile_pool(name="pheat", bufs=1, space="PSUM") as pheatp,
        ):
            # ---- constants / weights ----
            ones = constp.tile([128, 32], BF16)
            nc.vector.memset(ones[:], 1.0)

            wq_sb = wtp.tile([128, 2, 128], BF16)
            wk_sb = wtp.tile([128, 2, 128], BF16)
            wv_sb = wtp.tile([128, 2, 128], BF16)
            wp_sb = wtp.tile([64, 2, 2, 128], BF16)
            nc.sync.dma_start(wq_sb[:], wq.rearrange("(c p) m -> p c m", p=128))
            nc.sync.dma_start(wk_sb[:], wk.rearrange("(c p) m -> p c m", p=128))
            nc.sync.dma_start(wv_sb[:], wv.rearrange("(c p) m -> p c m", p=128))
            nc.sync.dma_start(
                wp_sb[:], wp.rearrange("r p (c m) -> p r c m", m=128))

            # ---- persistent big tiles ----
            xT = bigp.tile([128, 2, S], BF16)       # x^T: [ch, s]
            qT = bigp.tile([128, S], BF16)          # q^T: [(h d), s] scaled
            kT = bigp.tile([128, S], BF16)
            v_sb = bigp.tile([128, 25, 128], BF16)  # v: [s-chunk, (h d)]
            o_sb = bigp.tile([128, S], F32)         # attn out^T (unnormalized)
            m_sb = bigp.tile([128, S], F32)         # exp row-sums (replicated)
            o_nb = bigp.tile([64, 2, S], BF16)      # normalized out^T per pair

            # ---- phase 0: x^T via DMA transpose (bf16) ----
            for cc in range(2):
                nc.sync.dma_start_transpose(
                    xT[:, cc, :], x[:, cc * 128:(cc + 1) * 128])

            # ---- phase 1: qT, kT, v ----
            for qb in range(NQB):
                qs = slice(qb * QB, (qb + 1) * QB)
                for wsb, dst in ((wq_sb, qT), (wk_sb, kT)):
                    pt = poutp.tile([128, 512], F32, tag="out")
                    nc.tensor.matmul(
                        pt[:, 0:QB], wsb[:, 0, :], xT[:, 0, qs],
                        start=True, stop=False,
                    )
                    nc.tensor.matmul(
                        pt[:, 0:QB], wsb[:, 1, :], xT[:, 1, qs],
                        start=False, stop=True,
                    )
                    nc.vector.tensor_copy(out=dst[:, qs], in_=pt[:, 0:QB])
            off = 0
            for sc, rows in enumerate(KCH):
                pt = pscp.tile([128, 1024], F32, tag="sc", name=f"vp_{sc}")
                nc.tensor.matmul(
                    pt[:rows, 0:128], xT[:, 0, off:off + rows],
                    wv_sb[:, 0, :], start=True, stop=False,
                )
                nc.tensor.matmul(
                    pt[:rows, 0:128], xT[:, 1, off:off + rows],
                    wv_sb[:, 1, :], start=False, stop=True,
                )
                nc.vector.tensor_copy(
                    out=v_sb[:rows, sc, :], in_=pt[:rows, 0:128]
                )
                off += rows

            # ---- HAM heater ----
            # Tiled matmuls do not register as PE activity for the HAM
            # clock gate; a tiny plain-mode matmul once per batch keeps
            # the PE at 2.4 GHz. All heater matmuls accumulate into one
            # PSUM tile that is read out at the end (so none are dead).
            heat_ps = pheatp.tile([32, 64], F32)
            heat_n = [0]
            N_HEAT = 2 * NQB * 9

            def emit_heater():
                i = heat_n[0]
                heat_n[0] += 1
                nc.tensor.matmul(
                    heat_ps[:, :], ones[:, :], qT[:, 0:64],
                    start=(i == 0), stop=(i == N_HEAT - 1),
                    skip_group_check=True,
                )

            koffs = []
            koff = 0
            for krows in KCH:
                koffs.append(koff)
                koff += krows

            # ---- phase 2: attention, head pairs ----
            for p in range(2):
                pa = 64 * p          # partition base of head 2p in qT/kT
                pb = 64 * p + 32     # head 2p+1
                for qb in range(NQB):
                    qs = slice(qb * QB, (qb + 1) * QB)
                    out_ps = poutp.tile([128, 512], F32, tag="out")

                    def emit_scores_exp(kc, krows, koff):
                        ks = slice(koff, koff + krows)
                        sc_ps = pscp.tile([128, 1024], F32, tag="sc",
                                          name=f"sc_{p}_{qb}_{kc}")
                        # scores^T [k, q] for the two heads (row-packed)
                        nc.tensor.matmul(
                            sc_ps[:krows, 0:QB],
                            kT[pa:pa + 32, ks], qT[pa:pa + 32, qs],
                            start=True, stop=True, tile_position=(pa, 0),
                        )
                        nc.tensor.matmul(
                            sc_ps[:krows, 512:512 + QB],
                            kT[pb:pb + 32, ks], qT[pb:pb + 32, qs],
                            start=True, stop=True, tile_position=(pb, 0),
                        )
                        # exp (exact, ACT) PSUM -> SBUF
                        ex = expp.tile([128, 1024], BF16, tag="ex",
                                       name=f"ex_{p}_{qb}_{kc}")
                        sc3 = sc_ps[:krows].rearrange("p (b n) -> p b n", b=2)
                        ex3 = ex[:krows].rearrange("p (b n) -> p b n", b=2)
                        nc.scalar.activation(
                            ex3[:, :, 0:QB], sc3[:, :, 0:QB],
                            mybir.ActivationFunctionType.Exp,
                        )
                        return ex

                    def emit_attnv(kc, krows, ex):
                        st = kc == 0
                        sp = kc == len(KCH) - 1
                        nc.tensor.matmul(
                            out_ps[0:32, 0:QB],
                            v_sb[:krows, kc, 64 * p:64 * p + 32],
                            ex[:krows, 0:QB],
                            start=st, stop=sp, tile_position=(0, 0),
                        )
                        nc.tensor.matmul(
                            out_ps[32:64, 0:QB],
                            v_sb[:krows, kc, 64 * p + 32:64 * p + 64],
                            ex[:krows, 512:512 + QB],
                            start=st, stop=sp, tile_position=(0, 32),
                        )
                        nc.tensor.matmul(
                            out_ps[64:96, 0:QB],
                            ones[:krows, :], ex[:krows, 0:QB],
                            start=st, stop=sp, tile_position=(0, 64),
                        )
                        nc.tensor.matmul(
                            out_ps[96:128, 0:QB],
                            ones[:krows, :], ex[:krows, 512:512 + QB],
                            start=st, stop=sp, tile_position=(0, 96),
                        )

                    # software pipeline in batches of G chunks: one
                    # row-mode run (scores) + one col-mode run (attn@v)
                    # per batch amortizes the PE tiling-mode drains, and
                    # attn@v trails one batch so PE never waits on exp.
                    G = 3
                    batches = [
                        list(range(i, min(i + G, len(KCH))))
                        for i in range(0, len(KCH), G)
                    ]
                    pend = []
                    for bat in batches:
                        exs = [
                            emit_scores_exp(kc, KCH[kc], koffs[kc])
                            for kc in bat
                        ]
                        emit_heater()
                        for kc, ex in pend:
                            emit_attnv(kc, KCH[kc], ex)
                        pend = list(zip(bat, exs))
                    for kc, ex in pend:
                        emit_attnv(kc, KCH[kc], ex)
                    nc.vector.tensor_copy(
                        out=o_sb[pa:pa + 64, qs], in_=out_ps[0:64, 0:QB]
                    )
                    nc.vector.tensor_copy(
                        out=m_sb[pa:pa + 64, qs], in_=out_ps[64:128, 0:QB]
                    )

                # ---- per-pair normalize + projection partial ----
                nc.vector.reciprocal(
                    m_sb[pa:pa + 64, :], m_sb[pa:pa + 64, :])
                nc.vector.tensor_mul(
                    out=o_nb[:, p, :], in0=o_sb[pa:pa + 64, :],
                    in1=m_sb[pa:pa + 64, :])
                for qb in range(NQB):
                    qs = slice(qb * QB, (qb + 1) * QB)
                    for co in range(2):
                        pt = poutp.tile([128, 512], F32, tag="out")
                        nc.tensor.matmul(
                            pt[:, 0:QB], wp_sb[:, p, co, :], o_nb[:, p, qs],
                            start=True, stop=True,
                        )
                        ft = finp.tile([128, 512], F32, tag="fin")
                        nc.vector.tensor_copy(out=ft[:, 0:QB], in_=pt[:, 0:QB])
                        if p == 0:
                            nc.sync.dma_start(
                                outT[co * 128:(co + 1) * 128, qs],
                                ft[:, 0:QB])
                        else:
                            nc.gpsimd.dma_start(
                                outT[co * 128:(co + 1) * 128, qs],
                                ft[:, 0:QB], accum_op=mybir.AluOpType.add)

            # consume the heater chain so it is not dead code
            heat_sb = finp.tile([32, 64], F32, tag="heatsb")
            nc.vector.tensor_copy(out=heat_sb[:], in_=heat_ps[:, :])
            nc.sync.dma_start(heat[:], heat_sb[:])

    nc.compile()
    return nc


def _prep_in_maps(x, w_qkv, w_proj):
    bf = ml_dtypes.bfloat16
    x = np.asarray(x, dtype=np.float32)
    w_qkv = np.asarray(w_qkv, dtype=np.float32)
    w_proj = np.asarray(w_proj, dtype=np.float32)
    wq3 = w_qkv.reshape(DIM, 3, NUM_HEADS, HEAD_DIM)
    in_maps = []
    for core in range(8):
        b = core // 2
        hg = (core % 2) * HL
        wqc = np.ascontiguousarray(
            wq3[:, 0, hg:hg + HL, :].reshape(DIM, 128) * SCALE).astype(bf)
        wkc = np.ascontiguousarray(
            wq3[:, 1, hg:hg + HL, :].reshape(DIM, 128) * SCALE).astype(bf)
        wvc = np.ascontiguousarray(
            wq3[:, 2, hg:hg + HL, :].reshape(DIM, 128)).astype(bf)
        wpc = np.ascontiguousarray(
            w_proj[hg * HEAD_DIM:(hg + HL) * HEAD_DIM, :]
            .reshape(2, 64, DIM)).astype(bf)
        in_maps.append({
            "x": np.ascontiguousarray(x[b].reshape(S, DIM)).astype(bf),
            "wq": wqc, "wk": wkc, "wv": wvc, "wp": wpc,
        })
    return in_maps


def kernel(x, w_qkv, w_proj, _trace=False, _trace_kwargs=None):
    if "nc" not in _CACHE:
        _CACHE["nc"] = build_nc()
    nc = _CACHE["nc"]
    x = np.asarray(x, dtype=np.float32)
    in_maps = _prep_in_maps(x.reshape(B, S, DIM), w_qkv, w_proj)
    kw = {}
    if _trace:
        kw = dict(trace=True, **(_trace_kwargs or {}))
    res = run_bass_kernel_spmd(nc, in_maps, list(range(8)), **kw)
    _CACHE["last_result"] = res
    out = np.empty((B, S, DIM), dtype=np.float32)
    for b in range(B):
        acc = res.results[2 * b]["outT"] + res.results[2 * b + 1]["outT"]
        out[b] = acc.T
    return out.reshape(B, H, W, DIM)


# revision 9
# speedup vs baseline: 1.0303x; 1.0303x over previous
"""Bass/Trainium2 kernel for nn_Attention_37606733643961.

Full attention over B=4, S=56*56=3136, DIM=256, 8 heads of dim 32.
Sharding: 8 cores = (batch b, head-group g) with b = core//2, g = core%2.
Each core computes attention + projection-partial for its batch and its
4 heads; host sums the two projection partials per batch (row-split of
the c-contraction in the final matmul) and transposes back.

exp runs exactly on the Scalar engine (ACT) reading scores straight
from PSUM. Row-sums of exp ride the attn@v col-packed matmuls as
`ones` columns; normalization (reciprocal + multiply) happens
on-device before the projection. Matmuls are bf16 (fp32 PSUM accum).
A plain-mode "heater" matmul keeps the PE clock at 2.4 GHz (tiled
matmuls do not register as activity for the HAM clock gate).
"""

import sys

if "/opt/trn_rl_repo" not in sys.path:
    sys.path.insert(0, "/opt/trn_rl_repo")

import ml_dtypes
import numpy as np

import concourse.bacc as bacc
import concourse.mybir as mybir
from concourse.tile import TileContext
from concourse.bass_utils import run_bass_kernel_spmd

B, H, W, DIM = 4, 56, 56, 256
NUM_HEADS = 8
HEAD_DIM = 32
S = H * W                     # 3136
HL = 4                        # heads per core
SCALE = float(HEAD_DIM) ** -0.25

QB = 448                      # q-block (3136 = 7*448), <=512 (one PSUM bank)
NQB = S // QB                 # 7
KCH = [128] * 24 + [64]       # k-position chunks (3136 = 24*128 + 64)

F32 = mybir.dt.float32
BF16 = mybir.dt.bfloat16

_CACHE = {}


def build_nc():
    nc = bacc.Bacc(None, target_bir_lowering=False, debug=False)

    x = nc.dram_tensor("x", [S, DIM], BF16, kind="ExternalInput")
    wq = nc.dram_tensor("wq", [DIM, 128], BF16, kind="ExternalInput")
    wk = nc.dram_tensor("wk", [DIM, 128], BF16, kind="ExternalInput")
    wv = nc.dram_tensor("wv", [DIM, 128], BF16, kind="ExternalInput")
    # w_proj rows for this core's 4 heads, pre-split into head pairs
    wp = nc.dram_tensor("wp", [2, 64, DIM], BF16, kind="ExternalInput")
    outT = nc.dram_tensor("outT", [DIM, S], F32, kind="ExternalOutput")
    heat = nc.dram_tensor("heat", [32, 64], F32, kind="ExternalOutput")

    with TileContext(nc) as tc:
        with (
            tc.tile_pool(name="const", bufs=1) as constp,
            tc.tile_pool(name="wts", bufs=1) as wtp,
            tc.tile_pool(name="big", bufs=1) as bigp,
            tc.tile_pool(name="expst", bufs=8) as expp,
            tc.tile_pool(name="fin", bufs=3) as finp,
            tc.tile_pool(name="psc", bufs=3, space="PSUM") as pscp,
            tc.tile_pool(name="pout", bufs=1, space="PSUM") as poutp,
            tc.tile_pool(name="pheat", bufs=1, space="PSUM") as pheatp,
        ):
            # ---- constants / weights ----
            ones = constp.tile([128, 32], BF16)
            nc.vector.memset(ones[:], 1.0)

            wq_sb = wtp.tile([128, 2, 128], BF16)
            wk_sb = wtp.tile([128, 2, 128], BF16)
            wv_sb = wtp.tile([128, 2, 128], BF16)
            wp_sb = wtp.tile([64, 2, 2, 128], BF16)
            nc.sync.dma_start(wq_sb[:], wq.rearrange("(c p) m -> p c m", p=128))
            nc.sync.dma_start(wk_sb[:], wk.rearrange("(c p) m -> p c m", p=128))
            nc.sync.dma_start(wv_sb[:], wv.rearrange("(c p) m -> p c m", p=128))
            nc.sync.dma_start(
                wp_sb[:], wp.rearrange("r p (c m) -> p r c m", m=128))

            # ---- persistent big tiles ----
            xT = bigp.tile([128, 2, S], BF16)       # x^T: [ch, s]
            qT = bigp.tile([128, S], BF16)          # q^T: [(h d), s] scaled
            kT = bigp.tile([128, S], BF16)
            v_sb = bigp.tile([128, 25, 128], BF16)  # v: [s-chunk, (h d)]
            o_sb = bigp.tile([128, S], F32)         # attn out^T (unnormalized)
            m_sb = bigp.tile([128, S], F32)         # exp row-sums (replicated)
            o_nb = bigp.tile([64, 2, S], BF16)      # normalized out^T per pair

            # ---- phase 0: x^T via DMA transpose (bf16) ----
            for cc in range(2):
                nc.sync.dma_start_transpose(
                    xT[:, cc, :], x[:, cc * 128:(cc + 1) * 128])

            # ---- phase 1: qT, kT, v ----
            for qb in range(NQB):
                qs = slice(qb * QB, (qb + 1) * QB)
                for wsb, dst in ((wq_sb, qT), (wk_sb, kT)):
                    pt = poutp.tile([128, 512], F32, tag="out")
                    nc.tensor.matmul(
                        pt[:, 0:QB], wsb[:, 0, :], xT[:, 0, qs],
                        start=True, stop=False,
                    )
                    nc.tensor.matmul(
                        pt[:, 0:QB], wsb[:, 1, :], xT[:, 1, qs],
                        start=False, stop=True,
                    )
                    nc.vector.tensor_copy(out=dst[:, qs], in_=pt[:, 0:QB])
            off = 0
            for sc, rows in enumerate(KCH):
                pt = pscp.tile([128, 1024], F32, tag="sc", name=f"vp_{sc}")
                nc.tensor.matmul(
                    pt[:rows, 0:128], xT[:, 0, off:off + rows],
                    wv_sb[:, 0, :], start=True, stop=False,
                )
                nc.tensor.matmul(
                    pt[:rows, 0:128], xT[:, 1, off:off + rows],
                    wv_sb[:, 1, :], start=False, stop=True,
                )
                nc.vector.tensor_copy(
                    out=v_sb[:rows, sc, :], in_=pt[:rows, 0:128]
                )
                off += rows

            # ---- HAM heater ----
            # Tiled matmuls do not register as PE activity for the HAM
            # clock gate; a tiny plain-mode matmul once per batch keeps
            # the PE at 2.4 GHz. All heater matmuls accumulate into one
            # PSUM tile that is read out at the end (so none are dead).
            heat_ps = pheatp.tile([32, 64], F32)
            heat_n = [0]
            N_HEAT = 2 * NQB * ((len(KCH) + 2) // 3)

            def emit_heater():
                i = heat_n[0]
                heat_n[0] += 1
                nc.tensor.matmul(
                    heat_ps[:, :], ones[:, :], qT[:, 0:64],
                    start=(i == 0), stop=(i == N_HEAT - 1),
                    skip_group_check=True,
                )

            koffs = []
            koff = 0
            for krows in KCH:
                koffs.append(koff)
                koff += krows

            # ---- phase 2: attention — one flat software-pipelined
            # stream over (pair, q-block, k-batch) so the ACT exp never
            # stalls at q-block or pair boundaries.
            G = 3
            nbat = (len(KCH) + G - 1) // G
            batches = [
                list(range(i, min(i + G, len(KCH))))
                for i in range(0, len(KCH), G)
            ]

            def emit_scores_exp(p, qb, kc):
                pa = 64 * p
                pb = pa + 32
                qs = slice(qb * QB, (qb + 1) * QB)
                krows = KCH[kc]
                ks = slice(koffs[kc], koffs[kc] + krows)
                sc_ps = pscp.tile([128, 1024], F32, tag="sc",
                                  name=f"sc_{p}_{qb}_{kc}")
                nc.tensor.matmul(
                    sc_ps[:krows, 0:QB],
                    kT[pa:pa + 32, ks], qT[pa:pa + 32, qs],
                    start=True, stop=True, tile_position=(pa, 0),
                )
                nc.tensor.matmul(
                    sc_ps[:krows, 512:512 + QB],
                    kT[pb:pb + 32, ks], qT[pb:pb + 32, qs],
                    start=True, stop=True, tile_position=(pb, 0),
                )
                ex = expp.tile([128, 1024], BF16, tag="ex",
                               name=f"ex_{p}_{qb}_{kc}")
                sc3 = sc_ps[:krows].rearrange("p (b n) -> p b n", b=2)
                ex3 = ex[:krows].rearrange("p (b n) -> p b n", b=2)
                nc.scalar.activation(
                    ex3[:, :, 0:QB], sc3[:, :, 0:QB],
                    mybir.ActivationFunctionType.Exp,
                )
                return ex

            def emit_attnv(p, qb, out_ps, kc, ex):
                krows = KCH[kc]
                st = kc == 0
                sp = kc == len(KCH) - 1
                nc.tensor.matmul(
                    out_ps[0:32, 0:QB],
                    v_sb[:krows, kc, 64 * p:64 * p + 32],
                    ex[:krows, 0:QB],
                    start=st, stop=sp, tile_position=(0, 0),
                )
                nc.tensor.matmul(
                    out_ps[32:64, 0:QB],
                    v_sb[:krows, kc, 64 * p + 32:64 * p + 64],
                    ex[:krows, 512:512 + QB],
                    start=st, stop=sp, tile_position=(0, 32),
                )
                nc.tensor.matmul(
                    out_ps[64:96, 0:QB],
                    ones[:krows, :], ex[:krows, 0:QB],
                    start=st, stop=sp, tile_position=(0, 64),
                )
                nc.tensor.matmul(
                    out_ps[96:128, 0:QB],
                    ones[:krows, :], ex[:krows, 512:512 + QB],
                    start=st, stop=sp, tile_position=(0, 96),
                )

            def finish_qb(p, qb, out_ps):
                pa = 64 * p
                qs = slice(qb * QB, (qb + 1) * QB)
                nc.vector.tensor_copy(
                    out=o_sb[pa:pa + 64, qs], in_=out_ps[0:64, 0:QB])
                nc.vector.tensor_copy(
                    out=m_sb[pa:pa + 64, qs], in_=out_ps[64:128, 0:QB])
                if qb == NQB - 1:
                    nc.vector.reciprocal(
                        m_sb[pa:pa + 64, :], m_sb[pa:pa + 64, :])
                    nc.vector.tensor_mul(
                        out=o_nb[:, p, :], in0=o_sb[pa:pa + 64, :],
                        in1=m_sb[pa:pa + 64, :])

            items = [(p, qb, b) for p in range(2) for qb in range(NQB)
                     for b in range(nbat)]
            pend = []          # [(p, qb, out_ps, kc, ex), ...]
            cur_out = {}
            for (p, qb, b) in items:
                exs = [emit_scores_exp(p, qb, kc) for kc in batches[b]]
                emit_heater()
                done_qb = None
                for (pp, pqb, ops, kc, ex) in pend:
                    emit_attnv(pp, pqb, ops, kc, ex)
                    if kc == len(KCH) - 1:
                        done_qb = (pp, pqb, ops)
                if done_qb is not None:
                    finish_qb(*done_qb)
                    cur_out.pop((done_qb[0], done_qb[1]), None)
                key = (p, qb)
                if key not in cur_out:
                    cur_out[key] = poutp.tile(
                        [128, 512], F32, tag="out", name=f"out_{p}_{qb}")
                pend = [(p, qb, cur_out[key], kc, ex)
                        for kc, ex in zip(batches[b], exs)]
            done_qb = None
            for (pp, pqb, ops, kc, ex) in pend:
                emit_attnv(pp, pqb, ops, kc, ex)
                if kc == len(KCH) - 1:
                    done_qb = (pp, pqb, ops)
            finish_qb(*done_qb)

            # ---- phase 3: projection (both pairs), pipelined ----
            for qb in range(NQB):
                qs = slice(qb * QB, (qb + 1) * QB)
                for p in range(2):
                    for co in range(2):
                        pt = pscp.tile([128, 1024], F32, tag="sc",
                                       name=f"pj_{p}_{qb}_{co}")
                        nc.tensor.matmul(
                            pt[:, 0:QB], wp_sb[:, p, co, :], o_nb[:, p, qs],
                            start=True, stop=True,
                        )
                        ft = finp.tile([128, 512], F32, tag="fin")
                        if co == 0:
                            nc.vector.tensor_copy(
                                out=ft[:, 0:QB], in_=pt[:, 0:QB])
                        else:
                            nc.scalar.copy(
                                out=ft[:, 0:QB], in_=pt[:, 0:QB])
                        if p == 0:
                            nc.sync.dma_start(
                                outT[co * 128:(co + 1) * 128, qs],
                                ft[:, 0:QB])
                        else:
                            nc.gpsimd.dma_start(
                                outT[co * 128:(co + 1) * 128, qs],
                                ft[:, 0:QB], accum_op=mybir.AluOpType.add)

            # consume the heater chain so it is not dead code
            heat_sb = finp.tile([32, 64], F32, tag="heatsb")
            nc.vector.tensor_copy(out=heat_sb[:], in_=heat_ps[:, :])
            nc.sync.dma_start(heat[:], heat_sb[:])

    nc.compile()
    return nc


def _prep_in_maps(x, w_qkv, w_proj):
    bf = ml_dtypes.bfloat16
    x = np.asarray(x, dtype=np.float32)
    w_qkv = np.asarray(w_qkv, dtype=np.float32)
    w_proj = np.asarray(w_proj, dtype=np.float32)
    wq3 = w_qkv.reshape(DIM, 3, NUM_HEADS, HEAD_DIM)
    in_maps = []
    for core in range(8):
        b = core // 2
        hg = (core % 2) * HL
        wqc = np.ascontiguousarray(
            wq3[:, 0, hg:hg + HL, :].reshape(DIM, 128) * SCALE).astype(bf)
        wkc = np.ascontiguousarray(
            wq3[:, 1, hg:hg + HL, :].reshape(DIM, 128) * SCALE).astype(bf)
        wvc = np.ascontiguousarray(
            wq3[:, 2, hg:hg + HL, :].reshape(DIM, 128)).astype(bf)
        wpc = np.ascontiguousarray(
            w_proj[hg * HEAD_DIM:(hg + HL) * HEAD_DIM, :]
            .reshape(2, 64, DIM)).astype(bf)
        in_maps.append({
            "x": np.ascontiguousarray(x[b].reshape(S, DIM)).astype(bf),
            "wq": wqc, "wk": wkc, "wv": wvc, "wp": wpc,
        })
    return in_maps


def kernel(x, w_qkv, w_proj, _trace=False, _trace_kwargs=None):
    if "nc" not in _CACHE:
        _CACHE["nc"] = build_nc()
    nc = _CACHE["nc"]
    x = np.asarray(x, dtype=np.float32)
    in_maps = _prep_in_maps(x.reshape(B, S, DIM), w_qkv, w_proj)
    kw = {}
    if _trace:
        kw = dict(trace=True, **(_trace_kwargs or {}))
    res = run_bass_kernel_spmd(nc, in_maps, list(range(8)), **kw)
    _CACHE["last_result"] = res
    out = np.empty((B, S, DIM), dtype=np.float32)
    for b in range(B):
        acc = res.results[2 * b]["outT"] + res.results[2 * b + 1]["outT"]
        out[b] = acc.T
    return out.reshape(B, H, W, DIM)


# revision 10
# speedup vs baseline: 1.1418x; 1.1082x over previous
"""Bass/Trainium2 kernel for nn_Attention_37606733643961.

Full attention over B=4, S=56*56=3136, DIM=256, 8 heads of dim 32.
Sharding: 8 cores = (batch b, head-group g) with b = core//2, g = core%2.
Each core computes attention + projection-partial for its batch and its
4 heads; host sums the two projection partials per batch (row-split of
the c-contraction in the final matmul) and transposes back.

exp runs exactly on the Scalar engine (ACT) reading scores straight
from PSUM. Row-sums of exp ride the attn@v col-packed matmuls as
`ones` columns; normalization (reciprocal + multiply) happens
on-device before the projection. Matmuls are bf16 (fp32 PSUM accum).
A plain-mode "heater" matmul keeps the PE clock at 2.4 GHz (tiled
matmuls do not register as activity for the HAM clock gate).
"""

import sys

if "/opt/trn_rl_repo" not in sys.path:
    sys.path.insert(0, "/opt/trn_rl_repo")

import ml_dtypes
import numpy as np

import concourse.bacc as bacc
import concourse.mybir as mybir
from concourse.tile import TileContext
from concourse.bass_utils import run_bass_kernel_spmd

B, H, W, DIM = 4, 56, 56, 256
NUM_HEADS = 8
HEAD_DIM = 32
S = H * W                     # 3136
HL = 4                        # heads per core
SCALE = float(HEAD_DIM) ** -0.25

QB = 448                      # q-block (3136 = 7*448), <=512 (one PSUM bank)
NQB = S // QB                 # 7
KCH = [128] * 24 + [64]       # k-position chunks (3136 = 24*128 + 64)

F32 = mybir.dt.float32
BF16 = mybir.dt.bfloat16

_CACHE = {}


def build_nc():
    nc = bacc.Bacc(None, target_bir_lowering=False, debug=False)

    x = nc.dram_tensor("x", [S, DIM], BF16, kind="ExternalInput")
    wq = nc.dram_tensor("wq", [DIM, 128], BF16, kind="ExternalInput")
    wk = nc.dram_tensor("wk", [DIM, 128], BF16, kind="ExternalInput")
    wv = nc.dram_tensor("wv", [DIM, 128], BF16, kind="ExternalInput")
    # w_proj rows for this core's 4 heads, pre-split into head pairs
    wp = nc.dram_tensor("wp", [2, 64, DIM], BF16, kind="ExternalInput")
    outT = nc.dram_tensor("outT", [DIM, S], F32, kind="ExternalOutput")
    heat = nc.dram_tensor("heat", [32, 64], F32, kind="ExternalOutput")

    with TileContext(nc) as tc:
        with (
            tc.tile_pool(name="const", bufs=1) as constp,
            tc.tile_pool(name="wts", bufs=1) as wtp,
            tc.tile_pool(name="big", bufs=1) as bigp,
            tc.tile_pool(name="expst", bufs=8) as expp,
            tc.tile_pool(name="fin", bufs=3) as finp,
            tc.tile_pool(name="psc", bufs=3, space="PSUM") as pscp,
            tc.tile_pool(name="pout", bufs=1, space="PSUM") as poutp,
            tc.tile_pool(name="pheat", bufs=1, space="PSUM") as pheatp,
        ):
            # ---- constants / weights ----
            ones = constp.tile([128, 32], BF16)
            nc.vector.memset(ones[:], 1.0)

            wq_sb = wtp.tile([128, 2, 128], BF16)
            wk_sb = wtp.tile([128, 2, 128], BF16)
            wv_sb = wtp.tile([128, 2, 128], BF16)
            wp_sb = wtp.tile([64, 2, 2, 128], BF16)
            nc.sync.dma_start(wq_sb[:], wq.rearrange("(c p) m -> p c m", p=128))
            nc.sync.dma_start(wk_sb[:], wk.rearrange("(c p) m -> p c m", p=128))
            nc.sync.dma_start(wv_sb[:], wv.rearrange("(c p) m -> p c m", p=128))
            nc.sync.dma_start(
                wp_sb[:], wp.rearrange("r p (c m) -> p r c m", m=128))

            # ---- persistent big tiles ----
            xT = bigp.tile([128, 2, S], BF16)       # x^T: [ch, s]
            qT = bigp.tile([128, S], BF16)          # q^T: [(h d), s] scaled
            kT = bigp.tile([128, S], BF16)
            v_sb = bigp.tile([128, 25, 128], BF16)  # v: [s-chunk, (h d)]
            o_sb = bigp.tile([128, S], F32)         # attn out^T (unnormalized)
            m_sb = bigp.tile([128, S], F32)         # exp row-sums (replicated)
            o_nb = bigp.tile([64, 2, S], BF16)      # normalized out^T per pair

            # ---- phase 0: x^T via DMA transpose (bf16) ----
            for cc in range(2):
                nc.sync.dma_start_transpose(
                    xT[:, cc, :], x[:, cc * 128:(cc + 1) * 128])

            # ---- phase 1: qT, kT, v ----
            for qb in range(NQB):
                qs = slice(qb * QB, (qb + 1) * QB)
                for wsb, dst in ((wq_sb, qT), (wk_sb, kT)):
                    pt = poutp.tile([128, 512], F32, tag="out")
                    nc.tensor.matmul(
                        pt[:, 0:QB], wsb[:, 0, :], xT[:, 0, qs],
                        start=True, stop=False,
                    )
                    nc.tensor.matmul(
                        pt[:, 0:QB], wsb[:, 1, :], xT[:, 1, qs],
                        start=False, stop=True,
                    )
                    if qb % 2 == 0:
                        nc.vector.tensor_copy(out=dst[:, qs], in_=pt[:, 0:QB])
                    else:
                        nc.scalar.copy(out=dst[:, qs], in_=pt[:, 0:QB])
            off = 0
            for sc, rows in enumerate(KCH):
                pt = pscp.tile([128, 1024], F32, tag="sc", name=f"vp_{sc}")
                nc.tensor.matmul(
                    pt[:rows, 0:128], xT[:, 0, off:off + rows],
                    wv_sb[:, 0, :], start=True, stop=False,
                )
                nc.tensor.matmul(
                    pt[:rows, 0:128], xT[:, 1, off:off + rows],
                    wv_sb[:, 1, :], start=False, stop=True,
                )
                if sc % 2 == 0:
                    nc.vector.tensor_copy(
                        out=v_sb[:rows, sc, :], in_=pt[:rows, 0:128])
                else:
                    nc.scalar.copy(
                        out=v_sb[:rows, sc, :], in_=pt[:rows, 0:128])
                off += rows

            # ---- HAM heater ----
            # Tiled matmuls do not register as PE activity for the HAM
            # clock gate; a tiny plain-mode matmul once per batch keeps
            # the PE at 2.4 GHz. All heater matmuls accumulate into one
            # PSUM tile that is read out at the end (so none are dead).
            heat_ps = pheatp.tile([32, 64], F32)
            heat_n = [0]
            N_HEAT = 2 * NQB * ((len(KCH) + 2) // 3)

            def emit_heater():
                i = heat_n[0]
                heat_n[0] += 1
                nc.tensor.matmul(
                    heat_ps[:, :], ones[:, :], qT[:, 0:64],
                    start=(i == 0), stop=(i == N_HEAT - 1),
                    skip_group_check=True,
                )

            koffs = []
            koff = 0
            for krows in KCH:
                koffs.append(koff)
                koff += krows

            # ---- phase 2: attention — one flat software-pipelined
            # stream over (pair, q-block, k-batch) so the ACT exp never
            # stalls at q-block or pair boundaries.
            G = 3
            nbat = (len(KCH) + G - 1) // G
            batches = [
                list(range(i, min(i + G, len(KCH))))
                for i in range(0, len(KCH), G)
            ]

            def emit_scores_exp(p, qb, kc):
                pa = 64 * p
                pb = pa + 32
                qs = slice(qb * QB, (qb + 1) * QB)
                krows = KCH[kc]
                ks = slice(koffs[kc], koffs[kc] + krows)
                sc_ps = pscp.tile([128, 1024], F32, tag="sc",
                                  name=f"sc_{p}_{qb}_{kc}")
                nc.tensor.matmul(
                    sc_ps[:krows, 0:QB],
                    kT[pa:pa + 32, ks], qT[pa:pa + 32, qs],
                    start=True, stop=True, tile_position=(pa, 0),
                )
                nc.tensor.matmul(
                    sc_ps[:krows, 512:512 + QB],
                    kT[pb:pb + 32, ks], qT[pb:pb + 32, qs],
                    start=True, stop=True, tile_position=(pb, 0),
                )
                ex = expp.tile([128, 1024], BF16, tag="ex",
                               name=f"ex_{p}_{qb}_{kc}")
                sc3 = sc_ps[:krows].rearrange("p (b n) -> p b n", b=2)
                ex3 = ex[:krows].rearrange("p (b n) -> p b n", b=2)
                nc.scalar.activation(
                    ex3[:, :, 0:QB], sc3[:, :, 0:QB],
                    mybir.ActivationFunctionType.Exp,
                )
                return ex

            def emit_attnv(p, qb, out_ps, kc, ex):
                krows = KCH[kc]
                st = kc == 0
                sp = kc == len(KCH) - 1
                nc.tensor.matmul(
                    out_ps[0:32, 0:QB],
                    v_sb[:krows, kc, 64 * p:64 * p + 32],
                    ex[:krows, 0:QB],
                    start=st, stop=sp, tile_position=(0, 0),
                )
                nc.tensor.matmul(
                    out_ps[32:64, 0:QB],
                    v_sb[:krows, kc, 64 * p + 32:64 * p + 64],
                    ex[:krows, 512:512 + QB],
                    start=st, stop=sp, tile_position=(0, 32),
                )
                nc.tensor.matmul(
                    out_ps[64:96, 0:QB],
                    ones[:krows, :], ex[:krows, 0:QB],
                    start=st, stop=sp, tile_position=(0, 64),
                )
                nc.tensor.matmul(
                    out_ps[96:128, 0:QB],
                    ones[:krows, :], ex[:krows, 512:512 + QB],
                    start=st, stop=sp, tile_position=(0, 96),
                )

            def finish_qb(p, qb, out_ps):
                pa = 64 * p
                qs = slice(qb * QB, (qb + 1) * QB)
                nc.vector.tensor_copy(
                    out=o_sb[pa:pa + 64, qs], in_=out_ps[0:64, 0:QB])
                nc.vector.tensor_copy(
                    out=m_sb[pa:pa + 64, qs], in_=out_ps[64:128, 0:QB])
                if qb == NQB - 1:
                    nc.vector.reciprocal(
                        m_sb[pa:pa + 64, :], m_sb[pa:pa + 64, :])
                    nc.vector.tensor_mul(
                        out=o_nb[:, p, :], in0=o_sb[pa:pa + 64, :],
                        in1=m_sb[pa:pa + 64, :])

            items = [(p, qb, b) for p in range(2) for qb in range(NQB)
                     for b in range(nbat)]
            pend = []          # [(p, qb, out_ps, kc, ex), ...]
            cur_out = {}
            for (p, qb, b) in items:
                exs = [emit_scores_exp(p, qb, kc) for kc in batches[b]]
                emit_heater()
                done_qb = None
                for (pp, pqb, ops, kc, ex) in pend:
                    emit_attnv(pp, pqb, ops, kc, ex)
                    if kc == len(KCH) - 1:
                        done_qb = (pp, pqb, ops)
                if done_qb is not None:
                    finish_qb(*done_qb)
                    cur_out.pop((done_qb[0], done_qb[1]), None)
                key = (p, qb)
                if key not in cur_out:
                    cur_out[key] = poutp.tile(
                        [128, 512], F32, tag="out", name=f"out_{p}_{qb}")
                pend = [(p, qb, cur_out[key], kc, ex)
                        for kc, ex in zip(batches[b], exs)]
            done_qb = None
            for (pp, pqb, ops, kc, ex) in pend:
                emit_attnv(pp, pqb, ops, kc, ex)
                if kc == len(KCH) - 1:
                    done_qb = (pp, pqb, ops)
            finish_qb(*done_qb)

            # ---- phase 3: projection, pipelined; the two head-pairs
            # accumulate into the same PSUM tile so one plain DMA per
            # block writes the final partial.
            for qb in range(NQB):
                qs = slice(qb * QB, (qb + 1) * QB)
                for co in range(2):
                    pt = pscp.tile([128, 1024], F32, tag="sc",
                                   name=f"pj_{qb}_{co}")
                    nc.tensor.matmul(
                        pt[:, 0:QB], wp_sb[:, 0, co, :], o_nb[:, 0, qs],
                        start=True, stop=False,
                    )
                    nc.tensor.matmul(
                        pt[:, 0:QB], wp_sb[:, 1, co, :], o_nb[:, 1, qs],
                        start=False, stop=True,
                    )
                    ft = finp.tile([128, 512], F32, tag="fin")
                    if co == 0:
                        nc.vector.tensor_copy(
                            out=ft[:, 0:QB], in_=pt[:, 0:QB])
                    else:
                        nc.scalar.copy(
                            out=ft[:, 0:QB], in_=pt[:, 0:QB])
                    nc.sync.dma_start(
                        outT[co * 128:(co + 1) * 128, qs], ft[:, 0:QB])

            # consume the heater chain so it is not dead code
            heat_sb = finp.tile([32, 64], F32, tag="heatsb")
            nc.vector.tensor_copy(out=heat_sb[:], in_=heat_ps[:, :])
            nc.sync.dma_start(heat[:], heat_sb[:])

    nc.compile()
    return nc


def _prep_in_maps(x, w_qkv, w_proj):
    bf = ml_dtypes.bfloat16
    x = np.asarray(x, dtype=np.float32)
    w_qkv = np.asarray(w_qkv, dtype=np.float32)
    w_proj = np.asarray(w_proj, dtype=np.float32)
    wq3 = w_qkv.reshape(DIM, 3, NUM_HEADS, HEAD_DIM)
    in_maps = []
    for core in range(8):
        b = core // 2
        hg = (core % 2) * HL
        wqc = np.ascontiguousarray(
            wq3[:, 0, hg:hg + HL, :].reshape(DIM, 128) * SCALE).astype(bf)
        wkc = np.ascontiguousarray(
            wq3[:, 1, hg:hg + HL, :].reshape(DIM, 128) * SCALE).astype(bf)
        wvc = np.ascontiguousarray(
            wq3[:, 2, hg:hg + HL, :].reshape(DIM, 128)).astype(bf)
        wpc = np.ascontiguousarray(
            w_proj[hg * HEAD_DIM:(hg + HL) * HEAD_DIM, :]
            .reshape(2, 64, DIM)).astype(bf)
        in_maps.append({
            "x": np.ascontiguousarray(x[b].reshape(S, DIM)).astype(bf),
            "wq": wqc, "wk": wkc, "wv": wvc, "wp": wpc,
        })
    return in_maps


def kernel(x, w_qkv, w_proj, _trace=False, _trace_kwargs=None):
    if "nc" not in _CACHE:
        _CACHE["nc"] = build_nc()
    nc = _CACHE["nc"]
    x = np.asarray(x, dtype=np.float32)
    in_maps = _prep_in_maps(x.reshape(B, S, DIM), w_qkv, w_proj)
    kw = {}
    if _trace:
        kw = dict(trace=True, **(_trace_kwargs or {}))
    res = run_bass_kernel_spmd(nc, in_maps, list(range(8)), **kw)
    _CACHE["last_result"] = res
    out = np.empty((B, S, DIM), dtype=np.float32)
    for b in range(B):
        acc = res.results[2 * b]["outT"] + res.results[2 * b + 1]["outT"]
        out[b] = acc.T
    return out.reshape(B, H, W, DIM)
